# revision 17
# baseline (speedup 1.0000x reference)
"""Distributed Trainium2 Bass kernel for nn_ClosedFlyLoop (v2).

Strategy (8 NeuronCores, shard X into 8 blocks of 256):
 - host: symmetrize v, split y into (m[4], s), pad X edge-replicate by H=25,
   per-core slab [7, 1024, 306] bf16.  The AP_CUT mask is folded into each
   core's stage-4b band constants (per-core band data, same program).
 - device, per core (no collectives):
   1a: conv-y (circular 25-tap) as banded matmuls transposing [y,x]->[x,y'],
       one 2-bank psum + one evac per (channel, x-chunk); channel-paired
       partition-stacking for the 50-wide 3rd chunk.  gyk gets circular wrap
       columns so stage-1b can read y'+-1 shifted views.
   1b: per y-tile, conv-x banded matmuls transposing [x,y']->[y',x'] build a
       12-field gradient set in 3 psum quads; d/dy rides +-1-shifted lhsT
       views with sign-flipped bands (psum accumulates the central diff);
       w and trE accumulate directly in psum.  3 quad evacs -> bf16 grads.
   alg: batched bf16 pointwise algebra (quint/quad ops, scalar_tensor_tensor
       fusions) -> 5 pre fields.
   4a/4b: final smooth via the same two banded-matmul passes; mask folded in
       4b bands; per-field single evac; bf16 outputs.
 - host: concat per-core [5, 1024, 256] outputs, upcast f32.
"""
import numpy as np
import ml_dtypes

import concourse.bass as bass
import concourse.bacc as bacc
import concourse.mybir as mybir
from concourse import tile
from concourse.bass_utils import run_bass_kernel_spmd

BF16 = ml_dtypes.bfloat16
F32 = np.float32

Y, X = 1024, 2048
NCORES = 8
XS = X // NCORES            # 256
RAD = 12                    # gauss radius: int(4.0*3.0+0.5)
H = 2 * RAD + 1             # 25
W_IN = XS + 2 * H           # 306
W_ALG = XS + 2 * RAD        # 280
OFF1B = H - RAD             # 13: slab-coord offset of alg window
AP_CUT = 15
YT = Y // 128               # 8 y tiles
XT_IN = [(0, 128), (128, 128), (256, W_IN - 256)]       # slab x chunks (128,128,50)
XT_ALG = [(0, 128), (128, 128), (256, W_ALG - 256)]     # alg x chunks (128,128,24)
HALF = 512
GY_W = 1026                 # gyk free width: wrap col + 1024 + wrap col

# channel order: m00 m01 m10 m11 s v0 v1
# 1a psum/evac grouping: full chunks per channel; 3rd chunks channel-paired
PAIRS = [(0, 1), (2, 3), (4, 5), (6, None)]
PAIR_OF = {}
for pi, (ca, cb) in enumerate(PAIRS):
    PAIR_OF[ca] = (pi, 0)
    if cb is not None:
        PAIR_OF[cb] = (pi, 64)

# reaction constants
C_A = 13.9454545455         # 0.767/0.055
C_B = -1.2406779661         # -(0.732/0.59)
C_Q = -0.1968220339         # -(1.4375 - 1.2406779661)
C_Z = -0.0932203390         # 0.055/-0.59


def _gauss():
    r = RAD
    x = np.arange(-r, r + 1, dtype=np.float64)
    k = np.exp(-0.5 * (x / 3.0) ** 2)
    k = (k / k.sum())
    dk = np.convolve(k, [-0.5, 0.0, 0.5])
    return k, dk


KERN64, DKERN64 = _gauss()


# ---------------- band submatrix machinery (host) ----------------
class BandPack:
    """Dedup + pack band submatrices into one [128, K] bf16 constant.
    Blocks added with dedup=False always get fresh space (per-core values)."""

    def __init__(self):
        self.blocks = {}
        self.cols = []
        self.total = 0

    def add(self, sub, dedup=True, row_off=0):
        sub16 = np.ascontiguousarray(sub.astype(BF16))
        key = (sub16.shape, row_off, sub16.tobytes())
        if dedup and key in self.blocks:
            return self.blocks[key]
        pad = np.zeros((128, sub16.shape[1]), dtype=BF16)
        pad[row_off:row_off + sub16.shape[0]] = sub16
        off = self.total
        if dedup:
            self.blocks[key] = off
        self.cols.append(pad)
        self.total += sub16.shape[1]
        return off

    def packed(self):
        return np.concatenate(self.cols, axis=1)


def band_subs_y(pack, ker, scale):
    """Circular conv along y (1024). Per half h: list of (k, a, b, off, rows)."""
    r = len(ker) // 2
    B = np.zeros((Y, Y), dtype=np.float64)
    for j in range(Y):
        for t in range(-r, r + 1):
            B[(j + t) % Y, j] = ker[r + t] * scale
    out = []
    for h in range(2):
        subs = []
        for k in range(YT):
            sub = B[k * 128:(k + 1) * 128, h * HALF:(h + 1) * HALF]
            cols = np.flatnonzero(np.any(sub != 0.0, axis=0))
            if cols.size == 0:
                continue
            a, b = int(cols[0]), int(cols[-1] + 1)
            assert b - a == cols.size
            off = pack.add(sub[:, a:b])
            subs.append((k, a, b, off, 128))
        out.append(subs)
    return out


def band_matrix_x(ker, scale, n_in, n_out, off_in, mask=None):
    r = len(ker) // 2
    B = np.zeros((n_in, n_out), dtype=np.float64)
    for j in range(n_out):
        for t in range(-r, r + 1):
            i = j + off_in + t
            if 0 <= i < n_in:
                B[i, j] = ker[r + t] * scale
    if mask is not None:
        B *= mask[:, None]
    return B


def band_subs_x(pack, B, dedup=True, hi64=False):
    """Chunked subs of an [n_in, n_out] band: list of (k, a, b, off, rows).
    With hi64, partial (<128-row) chunks also get a copy placed at partition
    offset 64 (for pair-stacked lhsT tiles); entries become (k,a,b,(off,off64),rows)."""
    n_in = B.shape[0]
    subs = []
    nchunks = (n_in + 127) // 128
    for k in range(nchunks):
        rows = min(128, n_in - k * 128)
        sub = B[k * 128:k * 128 + rows, :]
        cols = np.flatnonzero(np.any(sub != 0.0, axis=0))
        if cols.size == 0:
            continue
        a, b = int(cols[0]), int(cols[-1] + 1)
        assert b - a == cols.size
        off = pack.add(sub[:, a:b], dedup=dedup)
        if hi64 and rows < 128:
            off64 = pack.add(sub[:, a:b], dedup=dedup, row_off=64)
            off = (off, off64)
        subs.append((k, a, b, off, rows))
    return subs


def build_bands():
    """Returns (subs dict, packed core0 bands, kx4 writeback info)."""
    pack = BandPack()
    S = {}
    S["ky"] = band_subs_y(pack, KERN64, 1.0)
    # conv-x bands: dy via +-shifted lhsT pairs (psum does the diff)
    for sc in (-0.5, 0.5, 0.25, -0.25):
        S[("kx", sc)] = band_subs_x(
            pack, band_matrix_x(KERN64, sc, W_IN, W_ALG, OFF1B), hi64=True)
    for sc in (-1.0, -0.5, 1.0):
        S[("dkx", sc)] = band_subs_x(
            pack, band_matrix_x(DKERN64, sc, W_IN, W_ALG, OFF1B), hi64=True)
    # stage 4b: Kx with per-core mask folded in; dedup off so the block is
    # private and can be overwritten per core.
    B4 = band_matrix_x(KERN64, 1.0, W_ALG, XS, RAD)
    S["kx4"] = band_subs_x(pack, B4, dedup=False)
    bands0 = pack.packed()
    return S, bands0


def core_bands(S, bands0, core):
    """Per-core band constant: core0 layout with kx4 blocks mask-folded."""
    b = bands0.copy()
    x0 = core * XS
    g = x0 + np.arange(W_ALG) - RAD
    mask = ((g >= AP_CUT) & (g < X - AP_CUT)).astype(np.float64)
    B4 = band_matrix_x(KERN64, 1.0, W_ALG, XS, RAD, mask=mask)
    for (k, a, bb, off, rows) in S["kx4"]:
        sub = B4[k * 128:k * 128 + rows, a:bb]
        blk = np.zeros((128, bb - a), dtype=BF16)
        blk[:rows] = sub.astype(BF16)
        b[:, off:off + bb - a] = blk
    return b


# dy plus/minus band scales per channel (band applied to y'+-1 shifted gyk)
DY_SC = {0: -0.5, 1: -0.5, 2: -0.5, 3: -0.5, 4: -0.5, 5: 0.5, 6: 0.25}
DX_SC = {0: -1.0, 1: -1.0, 2: -1.0, 3: -1.0, 4: -1.0, 5: -0.5, 6: 1.0}

AF = None  # set lazily (mybir.ActivationFunctionType)


def build_graph():
    S, bands0 = build_bands()
    KTOT = bands0.shape[1]

    nc = bacc.Bacc()
    x_ext = nc.declare_dram_parameter("x", [7, Y, W_IN], mybir.dt.bfloat16, isOutput=False)
    bands_ext = nc.declare_dram_parameter("bands", [128, KTOT], mybir.dt.bfloat16, isOutput=False)
    out_ext = nc.declare_dram_parameter("out", [5, Y, XS], mybir.dt.bfloat16, isOutput=True)

    bf = mybir.dt.bfloat16
    f32 = mybir.dt.float32
    TT = mybir.AluOpType
    Copy = mybir.ActivationFunctionType.Copy

    with tile.TileContext(nc) as tc:
        with (
            tc.tile_pool(name="const", bufs=1) as constp,
            tc.tile_pool(name="slab", bufs=1) as slabp,
            tc.tile_pool(name="gyt", bufs=1) as gytp,
            tc.tile_pool(name="grd", bufs=3) as grdp,
            tc.tile_pool(name="alg", bufs=3) as algp,
            tc.tile_pool(name="pre", bufs=1) as prep,
            tc.tile_pool(name="gyt2", bufs=2) as gyt2p,
            tc.tile_pool(name="outs", bufs=2) as outsp,
            tc.tile_pool(name="ps", bufs=4, space=bass.MemorySpace.PSUM) as psp,
        ):
            bands = constp.tile([128, KTOT], bf, tag="bands", name="bands")
            nc.sync.dma_start(bands[:, :], bands_ext[:, :])

            # slab: one tensor [128, 7, YT, W_IN]; per-channel DMA for pipelining
            slabT = slabp.tile([128, 7, YT, W_IN], bf, tag="slabT", name="slabT")
            for c in range(7):
                nc.sync.dma_start(
                    slabT[:, c, :, :],
                    x_ext[c].rearrange("(t p) x -> p t x", p=128))

            def duo():
                return psp.tile([128, 1024], f32, tag="h", name="h")

            # ---- hoisted slab-only algebra (runs during 1a window) ----
            # H5 slots: 0 u1, 1 u2n, 2 c1x, 3 b, 4 q  (per y-tile, persistent)
            Asl = slice(OFF1B, OFF1B + W_ALG)
            H5 = prep.tile([128, 5, YT, W_ALG], bf, tag="H5", name="H5")
            for t in range(YT):
                mv = [slabT[:, c, t, Asl] for c in range(4)]
                sv = slabT[:, 4, t, Asl]
                trm = algp.tile([128, W_ALG], bf, tag="trm", name="trm")
                nc.gpsimd.tensor_tensor(trm[:, :], mv[0], mv[3], TT.add)
                nc.gpsimd.tensor_tensor(H5[:, 0, t, :], mv[1], mv[2], TT.add)
                nc.gpsimd.tensor_tensor(H5[:, 1, t, :], mv[0], mv[3], TT.subtract)
                nc.scalar.activation(H5[:, 2, t, :], sv, Copy, bias=-0.11, scale=0.099)
                nc.vector.scalar_tensor_tensor(H5[:, 3, t, :], sv, C_B, trm[:, :], TT.add, TT.mult)
                nc.vector.scalar_tensor_tensor(H5[:, 4, t, :], trm[:, :], C_Q, H5[:, 3, t, :], TT.mult, TT.add)

            # ---------------- stage 1a: conv-y for all channels ----------------
            # gyk[c]: [128, 3, GY_W]; cols 1..1025 = y' 0..1023, wrap cols 0/1025.
            # 3rd x-chunk pair-stacked: channel at PAIR_OF[c][1] row offset of
            # gyk3[pair] (rows 0:50 / 64:114).
            gyk = [gytp.tile([128, 2, GY_W], bf, tag=f"gyk{c}", name=f"gyk{c}")
                   for c in range(7)]
            gyk3 = [gytp.tile([128, GY_W], bf, tag=f"gyk3_{p}", name=f"gyk3_{p}")
                    for p in range(len(PAIRS))]

            def conv_y(ps_ap, lhsT_fn, first=True, last=True):
                for h in range(2):
                    subs = S["ky"][h]
                    n = len(subs)
                    for i, (k, a, b, off, rows) in enumerate(subs):
                        nc.tensor.matmul(
                            ps_ap[:, h * HALF + a:h * HALF + b],
                            lhsT_fn(k),
                            bands[:rows, off:off + b - a],
                            start=(first and i == 0),
                            stop=(last and i == n - 1),
                        )

            evac1a_ctr = [0]

            def evac1a(dst, src):
                # alternate ACT / DVE for load balance
                if evac1a_ctr[0] % 2 == 0:
                    nc.scalar.copy(dst, src)
                else:
                    nc.vector.tensor_copy(dst, src)
                evac1a_ctr[0] += 1

            for c in range(7):
                for xt in range(2):
                    x0, xw = XT_IN[xt]
                    ps = duo()
                    conv_y(ps[:, 0:1024], lambda k: slabT[:, c, k, x0:x0 + xw])
                    evac1a(gyk[c][:, xt, 1:1025], ps[:, 0:1024])
                    # circular wrap cols from freshly written sbuf
                    g = gyk[c][:, xt, :]
                    nc.vector.tensor_copy(g[:, 0:GY_W:1025], g[:, 1024:0:-1023])
            x0, xw = XT_IN[2]
            for pi, (ca, cb) in enumerate(PAIRS):
                ps = duo()
                conv_y(ps[0:xw, 0:1024], lambda k: slabT[:, ca, k, x0:x0 + xw],
                       last=(cb is None))
                if cb is not None:
                    conv_y(ps[64:64 + xw, 0:1024],
                           lambda k: slabT[:, cb, k, x0:x0 + xw], first=False)
                rows = xw if cb is None else 64 + xw
                evac1a(gyk3[pi][0:rows, 1:1025], ps[0:rows, 0:1024])
                g = gyk3[pi]
                nc.vector.tensor_copy(g[0:rows, 0:GY_W:1025], g[0:rows, 1024:0:-1023])

            # ---------------- stage 1b + algebra, per y-tile ----------------
            # grads slots: 0-3 dy_m, 4 dys, 5 dxs, 6-9 dx_m, 10 w, 11 trE
            pre = prep.tile([128, 5, YT, W_ALG], bf, tag="pre", name="pre")

            def conv_x(ps_ap, c, subs, t, shift, first, last):
                n = len(subs)
                lo = t * 128 + shift
                for i, (k, a, b, off, rows) in enumerate(subs):
                    if k < 2:
                        lh = gyk[c][:, k, 1 + lo:129 + lo]
                        po = 0
                    else:
                        pi, po = PAIR_OF[c]
                        lh = gyk3[pi][po:po + rows, 1 + lo:129 + lo]
                    if isinstance(off, tuple):
                        off = off[1] if po else off[0]
                    nc.tensor.matmul(
                        ps_ap[:, a:b],
                        lh,
                        bands[po:po + rows, off:off + b - a],
                        start=(first and i == 0),
                        stop=(last and i == n - 1),
                    )

            def dy_groups(ps_ap, c, t, first=True, last=True):
                """+-1 shifted Kx pair: psum accumulates the central y-diff."""
                sc = DY_SC[c]
                conv_x(ps_ap, c, S[("kx", sc)], t, +1, first, False)
                conv_x(ps_ap, c, S[("kx", -sc)], t, -1, False, last)

            def dx_group(ps_ap, c, t, first=True, last=True):
                conv_x(ps_ap, c, S[("dkx", DX_SC[c])], t, 0, first, last)

            for t in range(YT):
                grads = grdp.tile([128, 12, W_ALG], bf, tag="grads", name="grads")

                def duo2():
                    return duo().rearrange("p (b x) -> p b x", b=2)

                def gevac(slots, ps2):
                    nc.scalar.copy(grads[:, slots:slots + 2, :], ps2[:, :, 0:W_ALG])

                dA1 = duo2()                       # dy m0, m1
                dy_groups(dA1[:, 0, :], 0, t)
                dy_groups(dA1[:, 1, :], 1, t)
                gevac(0, dA1)
                dA2 = duo2()                       # dy m2, m3
                dy_groups(dA2[:, 0, :], 2, t)
                dy_groups(dA2[:, 1, :], 3, t)
                gevac(2, dA2)
                dB1 = duo2()                       # dys, dxs
                dy_groups(dB1[:, 0, :], 4, t)
                dx_group(dB1[:, 1, :], 4, t)
                gevac(4, dB1)
                dB2 = duo2()                       # w, trE
                dx_group(dB2[:, 0, :], 5, t, first=True, last=False)
                dy_groups(dB2[:, 0, :], 6, t, first=False, last=True)
                dy_groups(dB2[:, 1, :], 5, t, last=False)
                dx_group(dB2[:, 1, :], 6, t, first=False, last=True)
                gevac(10, dB2)
                dC1 = duo2()                       # dx m0, m1
                dx_group(dC1[:, 0, :], 0, t)
                dx_group(dC1[:, 1, :], 1, t)
                gevac(6, dC1)
                dC2 = duo2()                       # dx m2, m3
                dx_group(dC2[:, 0, :], 2, t)
                dx_group(dC2[:, 1, :], 3, t)
                gevac(8, dC2)

                # ---- algebra ----
                sv = slabT[:, 4, t, Asl]
                m4 = slabT[:, 0:4, t, Asl]
                wf = grads[:, 10, :]
                trE = grads[:, 11, :]
                u1 = H5[:, 0, t, :]
                u2n = H5[:, 1, t, :]
                c1x = H5[:, 2, t, :]
                b_ = H5[:, 3, t, :]
                q = H5[:, 4, t, :]

                def tmp(tag, shape=None):
                    return algp.tile(shape or [128, W_ALG], bf, tag=tag, name=tag)

                a = tmp("a"); z2 = tmp("z2"); Ac = tmp("Ac")
                nc.vector.scalar_tensor_tensor(a[:, :], sv, C_A, trE[:, :], TT.add, TT.mult)
                nc.vector.scalar_tensor_tensor(z2[:, :], a[:, :], C_Z, b_, TT.mult, TT.add)
                nc.vector.scalar_tensor_tensor(Ac[:, :], z2[:, :], -0.59, c1x, TT.mult, TT.add)

                WU = tmp("WU", [128, 4, W_ALG])
                nc.vector.tensor_tensor(WU[:, 3, :], wf[:, :], u1, TT.mult)
                nc.vector.tensor_scalar(WU[:, 0, :], WU[:, 3, :], -1.0, None, TT.mult)
                wb = wf[:, :].unsqueeze(1).broadcast_to([128, 2, W_ALG])
                ub = u2n.unsqueeze(1).broadcast_to([128, 2, W_ALG])
                nc.gpsimd.tensor_tensor(WU[:, 1:3, :], wb, ub, TT.mult)

                Q12 = tmp("Q12", [128, 2, 5, W_ALG])
                vb = slabT[:, 5:7, t, Asl].unsqueeze(2).broadcast_to([128, 2, 5, W_ALG])
                g10 = grads[:, 0:10, :].rearrange("p (g s) x -> p g s x", g=2)
                nc.vector.tensor_tensor(Q12[:, :, :, :], g10, vb, TT.mult)
                p5 = pre[:, :, t, :]
                nc.vector.tensor_tensor(p5, Q12[:, 0, :, :], Q12[:, 1, :, :], TT.add)

                R4 = tmp("R4", [128, 4, W_ALG])
                Ab = Ac[:, :].unsqueeze(1).broadcast_to([128, 4, W_ALG])
                nc.vector.tensor_tensor(R4[:, :, :], m4, Ab, TT.mult)
                p4 = pre[:, 0:4, t, :]
                nc.vector.tensor_tensor(p4, p4, R4[:, :, :], TT.add)
                nc.gpsimd.tensor_tensor(p4, p4, WU[:, :, :], TT.add)
                p0 = pre[:, 0, t, :]
                nc.vector.scalar_tensor_tensor(p0, q, -0.048, p0, TT.mult, TT.add)

            # ---------------- stage 4: final smooth of 5 fields ----------------
            for f in range(5):
                gy2 = gyt2p.tile([128, 3, 1024], bf, tag="gy2", name="gy2")
                for xt in range(3):
                    x0, xw = XT_ALG[xt]
                    ps = duo()
                    conv_y(ps[0:xw, 0:1024], lambda k: pre[:, f, k, x0:x0 + xw])
                    if xt == 2:
                        nc.vector.tensor_copy(gy2[0:xw, xt, :], ps[0:xw, 0:1024])
                    else:
                        nc.scalar.copy(gy2[0:xw, xt, :], ps[0:xw, 0:1024])

                ow = outsp.tile([128, YT, XS], bf, tag="ow", name="ow")
                for half in range(2):
                    ps4 = duo()
                    for tt in range(4):
                        t = half * 4 + tt
                        view = ps4[:, tt * 256: tt * 256 + XS]
                        subs = S["kx4"]
                        n = len(subs)
                        for i, (k, a, b, off, rows) in enumerate(subs):
                            if k < 2:
                                lh = gy2[:, k, t * 128:(t + 1) * 128]
                            else:
                                lh = gy2[0:rows, 2, t * 128:(t + 1) * 128]
                            nc.tensor.matmul(
                                view[:, a:b], lh, bands[:rows, off:off + b - a],
                                start=(i == 0), stop=(i == n - 1))
                    dst = ow[:, half * 4:half * 4 + 4, :]
                    src = ps4.rearrange("p (s x) -> p s x", s=4)
                    if half == 0:
                        nc.scalar.copy(dst, src)
                    else:
                        nc.vector.tensor_copy(dst, src)
                nc.sync.dma_start(
                    out_ext[f].rearrange("(t p) x -> p t x", p=128), ow[:, :, :])

    nc.compile()
    return nc, S, bands0


_CACHE = {}


def _get_graph():
    if "nc" not in _CACHE:
        _CACHE["nc"], _CACHE["S"], _CACHE["bands0"] = build_graph()
    return _CACHE["nc"], _CACHE["S"], _CACHE["bands0"]


def host_prep(y, v):
    m = y[:4]
    s = y[4:5]
    v_lr = v[:, ::-1, :].copy()
    v_lr[0] *= -1.0
    vs = 0.5 * (v + v_lr)
    f = np.concatenate([m, s, vs], axis=0).astype(F32)      # [7, Y, X]
    fp = np.pad(f, ((0, 0), (0, 0), (H, H)), mode='edge')
    slabs = []
    for c in range(NCORES):
        x0 = c * XS
        slabs.append(np.ascontiguousarray(fp[:, :, x0:x0 + W_IN]).astype(BF16))
    return slabs


def kernel(y, v):
    y = np.asarray(y, dtype=F32)
    v = np.asarray(v, dtype=F32)
    nc, S, bands0 = _get_graph()
    slabs = host_prep(y, v)
    in_maps = [
        {"x": slabs[c], "bands": core_bands(S, bands0, c)}
        for c in range(NCORES)
    ]
    res = run_bass_kernel_spmd(nc, in_maps, core_ids=list(range(NCORES)))
    out = np.concatenate(
        [np.asarray(res.results[c]["out"], dtype=F32) for c in range(NCORES)], axis=2)
    return out
